# revision 33
# baseline (speedup 1.0000x reference)
"""Multi-head causal attention (B=4, T=2048, C=1024, H=16) on 8 trn2 cores.

Sharding: core c handles batch b=c//2 and head-group hg=c%2 (8 heads).
Each core computes qkv projection for its heads, causal attention, and a
partial output projection; the host sums the two head-group partials per
batch. No collectives.

On-chip dataflow (per core, all fp16 operands / fp32 PSUM):
  x [T,C] --xbar-dma-transpose--> xT [C,T]
  qT/kT = (W.T x.T) feature-major [512, T]   (head-pair tiles [128, T])
  v     = (x W) natural [T, 512] interleaved with a ones column per head
  scores^T [k,q] per head pair via row-tiled K=64 matmul packing, the two
    heads' tiles packed side by side in one [128, 1024] 2-bank PSUM tile
  expS = one ACT exp(0.125*s) per [128,1024] tile -> fp16, causal mask on
    diagonal tiles only (mask input pre-doubled to 1024 wide)
  AV: psum[65, q] += v_aug[k,65].T @ expS[k,q]  (row 64 = softmax denom)
  o^T = numerator * bcast(1/denom)  folded into PSUM evacuation; the
    reciprocal runs on a DMA-packed [64,16] layout (64 lanes, not 1)
  out[t, :] += o^T.T @ w_proj  (natural layout, fp32 DMA out)
"""

import numpy as np

B, T, C, H = 4, 2048, 1024, 16
HD = C // H          # 64
HG = H // 2          # 8 heads per core
CPG = HG * HD        # 512 feature cols per head-group
NCORES = 8
NCT = C // 128       # 8 contraction tiles over C
NTT = T // 128       # 16 token tiles
NCH = T // 512       # 4 query chunks
VROW = HG * (HD + 1)  # 520: v row layout, 65 cols per head (64 v + 1 ones)

_cache = {}


def _build_nc(dump=False):
    import concourse.mybir as mybir
    import concourse.tile as tile
    from concourse import bacc

    f16 = mybir.dt.float16
    f32 = mybir.dt.float32
    mult = mybir.AluOpType.mult
    Exp = mybir.ActivationFunctionType.Exp

    nc = bacc.Bacc(None, target_bir_lowering=False, debug=False)

    xT_d = nc.dram_tensor("xT", [C, T], f16, kind="ExternalInput")
    wq_d = nc.dram_tensor("wq", [C, 3 * CPG], f16, kind="ExternalInput")
    wp_d = nc.dram_tensor("wp", [CPG, C], f16, kind="ExternalInput")
    mask_d = nc.dram_tensor("mask", [128, 256], f16, kind="ExternalInput")
    out_d = nc.dram_tensor("out", [T, C], f32, kind="ExternalOutput")
    dump_d = {}
    if dump:
        for name, cols in [("xT", NCT * T), ("qT", 4 * T), ("kT", 4 * T),
                           ("v_sb", NTT * VROW), ("oT", 4 * T)]:
            dump_d[name] = nc.dram_tensor(f"dump_{name}", [128, cols], f16,
                                          kind="ExternalOutput")

    with tile.TileContext(nc) as tc:
        with (
            tc.tile_pool(name="persist", bufs=1) as pp,
            tc.tile_pool(name="work", bufs=4) as wk,
            tc.tile_pool(name="psum", bufs=1, space="PSUM") as psp,
        ):
            # ---- persistent SBUF tensors ----
            xT = pp.tile([128, NCT * T], f16)          # [c-tile, t] 32KB/part
            w_sb = pp.tile([128, NCT * 3 * CPG], f16)  # qkv weights 24KB/part
            wp_sb = pp.tile([128, 4 * C], f16)         # proj weights 8KB/part
            mask_sb = pp.tile([128, 256], f16)
            qT = pp.tile([128, 4 * T], f16)            # 4 head-pair tiles
            kT = pp.tile([128, 4 * T], f16)
            v_sb = pp.tile([128, NTT * VROW], f16)     # v + ones cols
            oT = pp.tile([128, 4 * T], f16)            # attn out, pair tiles

            # ---- input DMA (x pre-transposed on host; spread both queues,
            # split in halves so the first matmuls start sooner) ----
            for ct in range(NCT):
                for h2 in range(2):
                    nc.gpsimd.dma_start(
                        out=w_sb[:, ct * 3 * CPG + h2 * 768:ct * 3 * CPG + (h2 + 1) * 768],
                        in_=wq_d[ct * 128:(ct + 1) * 128, h2 * 768:(h2 + 1) * 768],
                    )
                    nc.sync.dma_start(
                        out=xT[:, ct * T + h2 * 1024:ct * T + (h2 + 1) * 1024],
                        in_=xT_d[ct * 128:(ct + 1) * 128, h2 * 1024:(h2 + 1) * 1024],
                    )
            for cp in range(4):
                nc.gpsimd.dma_start(
                    out=wp_sb[:, cp * C:(cp + 1) * C],
                    in_=wp_d[cp * 128:(cp + 1) * 128, :],
                )
            nc.gpsimd.dma_start(out=mask_sb[:], in_=mask_d[:])

            # ones columns for the softmax-denominator trick
            ones_view = v_sb.rearrange("p (n d) -> p n d", d=HD + 1)[:, :, HD:HD + 1]
            nc.vector.memset(ones_view, 1.0)

            # ---- stage 1b first: v natural [t, 512] with ones interleave ----
            for tt in range(NTT):
                ps = psp.tile([128, 512], f32, tag="avmm", bufs=4)
                for ct in range(NCT):
                    nc.tensor.matmul(
                        ps[:],
                        lhsT=xT[:, ct * T + tt * 128:ct * T + tt * 128 + 128],
                        rhs=w_sb[:, ct * 3 * CPG + 2 * CPG:(ct + 1) * 3 * CPG],
                        start=(ct == 0), stop=(ct == NCT - 1),
                    )
                vdst = v_sb.rearrange("p (n d) -> p n d", d=HD + 1)[
                    :, tt * HG:(tt + 1) * HG, 0:HD
                ]
                nc.vector.tensor_scalar_mul(
                    vdst, ps[:].rearrange("p (h d) -> p h d", d=HD), 1.0
                )

            def emit_qk(pair):
                # qT / kT feature-major for one head pair (c'-tiles pair, 4+pair)
                for cpt in (pair, 4 + pair):
                    dst = qT if cpt < 4 else kT
                    for tch in range(NCH):
                        ps = psp.tile([128, 512], f32, tag="avmm", bufs=4, name="qkps")
                        for ct in range(NCT):
                            nc.tensor.matmul(
                                ps[:],
                                lhsT=w_sb[:, ct * 3 * CPG + cpt * 128:ct * 3 * CPG + cpt * 128 + 128],
                                rhs=xT[:, ct * T + tch * 512:ct * T + tch * 512 + 512],
                                start=(ct == 0), stop=(ct == NCT - 1),
                            )
                        nc.vector.tensor_scalar_mul(
                            dst[:, pair * T + tch * 512:pair * T + tch * 512 + 512], ps[:], 1.0
                        )

            # ---- stage 2: per pair: qk projection then attention (the
            # next pair's qk matmuls fill PE gaps in this pair's
            # ACT-gated attention pipeline) ----
            emit_qk(0)
            for p in range(4):
                hA, hB = 2 * p, 2 * p + 1
                for ci in range(NCH):
                    jmax = 4 * ci + 3
                    av_a = psp.tile([65, 512], f32, tag="avmm", bufs=4)
                    av_b = psp.tile([65, 512], f32, tag="avmm", bufs=4)
                    for j in range(jmax + 1):
                        s_ab = psp.tile([128, 1024], f32, tag="s", bufs=2)
                        nc.tensor.matmul(
                            s_ab[:, 0:512],
                            lhsT=kT[0:64, p * T + j * 128:p * T + j * 128 + 128],
                            rhs=qT[0:64, p * T + ci * 512:p * T + ci * 512 + 512],
                            start=True, stop=True,
                        )
                        nc.tensor.matmul(
                            s_ab[:, 512:1024],
                            lhsT=kT[64:128, p * T + j * 128:p * T + j * 128 + 128],
                            rhs=qT[64:128, p * T + ci * 512:p * T + ci * 512 + 512],
                            start=True, stop=True,
                        )
                        e_ab = wk.tile([128, 1024], f16, tag="e", bufs=6)
                        r = j - 4 * ci
                        if r < 0:
                            nc.scalar.activation(e_ab[:], s_ab[:], Exp, scale=0.125)
                        else:
                            # diagonal tile: only columns >= 128r are causally
                            # reachable; exp just those windows in both head
                            # halves, then one 128-wide triangular mask mult.
                            # Columns < 128r stay garbage but are never read
                            # (the AV matmuls start at column 128r).
                            c0 = 128 * r
                            s3 = s_ab.rearrange("p (h q) -> p h q", q=512)
                            e3 = e_ab.rearrange("p (h q) -> p h q", q=512)
                            nc.scalar.activation(
                                e3[:, :, c0:512], s3[:, :, c0:512], Exp, scale=0.125)
                            m3 = mask_sb.rearrange("p (h q) -> p h q", q=128)
                            nc.vector.scalar_tensor_tensor(
                                e3[:, :, c0:c0 + 128], e3[:, :, c0:c0 + 128], 1.0,
                                m3[:], op0=mult, op1=mult,
                            )
                        # diagonal tiles: columns < 128r are exact zeros after
                        # the mask, so the AV matmul can skip them (the j=0
                        # start matmul always covers the full width)
                        c0 = 128 * r if r > 0 else 0
                        nc.tensor.matmul(
                            av_a[:, c0:512],
                            lhsT=v_sb[:, j * VROW + hA * 65:j * VROW + hA * 65 + 65],
                            rhs=e_ab[:, c0:512],
                            start=(j == 0), stop=(j == jmax),
                        )
                        nc.tensor.matmul(
                            av_b[:, c0:512],
                            lhsT=v_sb[:, j * VROW + hB * 65:j * VROW + hB * 65 + 65],
                            rhs=e_ab[:, 512 + c0:1024],
                            start=(j == 0), stop=(j == jmax),
                        )
                    # evacuate: divide by denominator (psum row 64).
                    # Single-lane reciprocal is ~3.3us; pack the 1024 denoms
                    # into 64 lanes via DMA round-trip instead.
                    den = wk.tile([1, 1024], f32, tag="den", bufs=2)
                    nc.vector.tensor_scalar_mul(den[0:1, 0:512], av_a[64:65, :], 1.0)
                    nc.vector.tensor_scalar_mul(den[0:1, 512:1024], av_b[64:65, :], 1.0)
                    denp = wk.tile([64, 16], f32, tag="denp", bufs=2)
                    nc.sync.dma_start(out=denp[:], in_=den[:])
                    recp = wk.tile([64, 16], f32, tag="recp", bufs=2)
                    nc.vector.reciprocal(recp[:], denp[:])
                    recip0 = wk.tile([1, 1024], f32, tag="recip0", bufs=2)
                    nc.sync.dma_start(out=recip0[:], in_=recp[:])
                    rbc = wk.tile([64, 1024], f32, tag="rbc", bufs=2)
                    nc.gpsimd.partition_broadcast(rbc[0:64, 0:512], recip0[0:1, 0:512])
                    nc.gpsimd.partition_broadcast(rbc[0:64, 512:1024], recip0[0:1, 512:1024])
                    nc.vector.scalar_tensor_tensor(
                        oT[0:64, p * T + ci * 512:p * T + ci * 512 + 512],
                        av_a[0:64, :], 1.0, rbc[0:64, 0:512], op0=mult, op1=mult,
                    )
                    tmpb = wk.tile([64, 512], f16, tag="tmpb", bufs=2)
                    nc.vector.scalar_tensor_tensor(
                        tmpb[:], av_b[0:64, :], 1.0, rbc[0:64, 512:1024],
                        op0=mult, op1=mult,
                    )
                    # shift head-B rows to partitions 64-127 of the pair tile
                    nc.sync.dma_start(
                        out=oT[64:128, p * T + ci * 512:p * T + ci * 512 + 512],
                        in_=tmpb[:],
                    )
                if p < 3:
                    # lower priority than attention(p): fills PE gaps
                    emit_qk(p + 1)

            # ---- stage 3: output projection (natural [t, out]) ----
            for tt in range(NTT):
                for oc in range(2):
                    ps = psp.tile([128, 512], f32, tag="avmm", bufs=4)
                    for cp in range(4):
                        nc.tensor.matmul(
                            ps[:],
                            lhsT=oT[:, cp * T + tt * 128:cp * T + tt * 128 + 128],
                            rhs=wp_sb[:, cp * C + oc * 512:cp * C + oc * 512 + 512],
                            start=(cp == 0), stop=(cp == 3),
                        )
                    ot = wk.tile([128, 512], f32, tag="ostage", bufs=4)
                    nc.vector.tensor_scalar_mul(ot[:], ps[:], 1.0)
                    nc.sync.dma_start(
                        out=out_d[tt * 128:(tt + 1) * 128, oc * 512:(oc + 1) * 512],
                        in_=ot[:],
                    )

            if dump:
                for name, sb in [("xT", xT), ("qT", qT), ("kT", kT),
                                 ("v_sb", v_sb), ("oT", oT)]:
                    nc.sync.dma_start(out=dump_d[name][:], in_=sb[:])

    nc.compile()
    return nc


def get_nc():
    if "nc" not in _cache:
        _cache["nc"] = _build_nc()
    return _cache["nc"]


def make_mask():
    # single 128x128 lower-triangular mask (k <= q), doubled side by side so
    # one 3D-AP multiply covers both heads of a packed pair.
    k = np.arange(128)[:, None]
    q = np.arange(128)[None, :]
    m = (k <= q)
    return np.concatenate([m, m], axis=1).astype(np.float16)


def make_in_maps(x, w_qkv, w_proj):
    f16 = np.float16
    mask = make_mask()
    in_maps = []
    for c in range(NCORES):
        b, hg = c // 2, c % 2
        cols = np.concatenate([
            np.arange(hg * CPG, hg * CPG + CPG),
            np.arange(C + hg * CPG, C + hg * CPG + CPG),
            np.arange(2 * C + hg * CPG, 2 * C + hg * CPG + CPG),
        ])
        in_maps.append({
            "xT": np.ascontiguousarray(x[b].astype(f16).T),
            "wq": np.ascontiguousarray(w_qkv[:, cols]).astype(f16),
            "wp": np.ascontiguousarray(w_proj[hg * CPG:(hg + 1) * CPG, :]).astype(f16),
            "mask": mask,
        })
    return in_maps


def kernel(x, w_qkv, w_proj, **run_kwargs):
    from concourse.bass_utils import run_bass_kernel_spmd

    x = np.asarray(x)
    nc = get_nc()
    in_maps = make_in_maps(x, np.asarray(w_qkv), np.asarray(w_proj))
    res = run_bass_kernel_spmd(nc, in_maps, list(range(NCORES)), **run_kwargs)
    _cache["last_results"] = res
    out = np.empty((B, T, C), np.float32)
    for b in range(B):
        out[b] = res.results[2 * b]["out"] + res.results[2 * b + 1]["out"]
    return out
